# revision 6
# baseline (speedup 1.0000x reference)
"""Trainium2 Bass kernel for the ragged-sequence GP ELBO problem.

Math per sampled row g (N=65536 locations, M=64 ancestor window):
  innerCov/marginalVar need x_u = R^-1 u and x_e = R^-1 e_63 where R is the
  64x64 upper-tri window V[A(g),A(g)].  Both solved by batched backward
  substitution on the vector engine, 128 partitions x S=8 samples x 2 RHS
  lanes per partition.

Device layout (per core, B/8 = 2048 samples, T=2 super-tiles of 128*S):
  - V is re-laid host-side as a zero-padded [N, 64] row-major band (bf16):
    row j holds CSR row j (entries k = cols j..j+63), zero-padded.  The
    window for sample g is the contiguous span [64*(g-63), 64*(g+1)) - one
    8KB indirect gather per sample-slot; window entry (r, c) sits at
    64*r + (c - r), diagonal at 64*r (4B-aligned for bf16 2x mode).
  - Substitution step r: ONE tensor_tensor mult whose stream starts at the
    diagonal: X[r] is prefilled with -b[r]/d[r], so the first product is
    -b[r] and the rest are A[r,c]*x[c]; tensor_reduce(negate) then yields
    bracket = b[r] - sum(A x) directly; a small combine writes
    X[r] = bracket * dinv[r].
  - mean/mean_post/y interleaved host-side (fp32) for one 768B gather;
    U rows gathered from a bf16 copy of U_values (64 ending at crow_u[g+1]).
  - Samples with g < 63 (~0.1%) use host-built patch windows appended to the
    value arrays (identity rows / zero padding), so device code is uniform.
  - Epilogue (logdet, innerMean, norms, resid) runs on ACT + Pool engines,
    overlapping the next tile's substitution on DVE.
  - Per-core partial sums land in acc[P, 8]; host sums 8x128 partials and
    applies the closed-form tail.

Sharding: mini_indices split contiguously across the 8 cores (data
parallel, sorted within core for gather locality); value arrays replicated.

This walrus build caps semaphore waits at 1 per instruction (2 per
EventSemaphore); _split_multiwait spills excess waits onto standalone
EventSemaphore instructions after Tile scheduling.  Custom DVE ops are NOT
supported by this walrus ("ISA wrong length"), so only stock ops are used.
"""
import numpy as np
import ml_dtypes

import concourse.bass as bass
import concourse.mybir as mybir
import concourse.tile as tile
from concourse.bass import AP, IndirectOffsetOnAxis
from concourse.bass_utils import run_bass_kernel_spmd

M = 64
N = 65536
NCORES = 8
P = 128
S = 8            # samples per partition per super-tile
VSPAN = 64 * M   # 4096: contiguous window span in the padded [N,64] layout
F32 = mybir.dt.float32
BF16 = mybir.dt.bfloat16
I32 = mybir.dt.int32
BF_NP = ml_dtypes.bfloat16

_cache = {}


def _split_multiwait(nc):
    """Spill excess sync waits onto standalone EventSemaphores (this
    walrus allows 1 wait per instruction, 2 per EventSemaphore)."""
    for fn in nc.m.functions:
        for blk in fn.blocks:
            insts = blk.instructions
            newlist = []
            n_new = 0
            for ins in insts:
                si = ins.sync_info
                cap = 2 if isinstance(ins, mybir.InstEventSemaphore) else 1
                if si is not None and len(si.on_wait) > cap:
                    waits = list(si.on_wait)
                    spill, keep = waits[:-cap], waits[-cap:]
                    k = 0
                    while k < len(spill):
                        chunk = spill[k:k + 2]
                        k += 2
                        n_new += 1
                        ev = mybir.InstEventSemaphore(
                            name=f"{ins.name}_sw{k}",
                            engine=ins.engine,
                            ins=[], outs=[],
                            sync_info=mybir.SyncInfo(on_wait=chunk,
                                                     on_update=[]))
                        newlist.append(ev)
                    ins.sync_info = mybir.SyncInfo(
                        on_wait=keep, on_update=list(si.on_update))
                newlist.append(ins)
            if n_new:
                insts[:] = newlist
    return nc


VARIANT = dict(xdt='bf16',      # 'bf16' | 'f32' : X/products dtype
               joint=True,      # joint e/u mult (stride-0 axis) vs split
               dualx=False)     # shifted second X copy for odd-r alignment


def _build_program(T, NV2, NU2, NM3, split=True, reps=1):
    """Bass program for one core: T super-tiles of 128*S samples."""
    nc = bass.Bass()
    v2 = nc.declare_dram_parameter("v2", [NV2, 1], BF16, isOutput=False)
    u2 = nc.declare_dram_parameter("u2", [NU2, 1], BF16, isOutput=False)
    m3 = nc.declare_dram_parameter("m3", [NM3, 1], F32, isOutput=False)
    offs_v = nc.declare_dram_parameter("offs_v", [P, T * S], I32, isOutput=False)
    offs_u = nc.declare_dram_parameter("offs_u", [P, T * S], I32, isOutput=False)
    offs_m = nc.declare_dram_parameter("offs_m", [P, T * S], I32, isOutput=False)
    out = nc.declare_dram_parameter("out", [P, 8], F32, isOutput=True)

    mult = mybir.AluOpType.mult
    addop = mybir.AluOpType.add
    subop = mybir.AluOpType.subtract

    with tile.TileContext(nc) as tc:
        with (
            tc.tile_pool(name="pv", bufs=2) as pv,
            tc.tile_pool(name="pw", bufs=2) as pw,
            tc.tile_pool(name="ps", bufs=2) as ps,
            tc.tile_pool(name="pacc", bufs=1) as pacc,
        ):
            acc = pacc.tile([P, 8], F32)
            nc.scalar.memzero(acc[:])
            ov_all = pacc.tile([P, T * S], I32)
            nc.sync.dma_start(out=ov_all[:], in_=offs_v[:, :])
            ou_all = pacc.tile([P, T * S], I32)
            nc.sync.dma_start(out=ou_all[:], in_=offs_u[:, :])
            om_all = pacc.tile([P, T * S], I32)
            nc.sync.dma_start(out=om_all[:], in_=offs_m[:, :])

            def accslot(q):
                a = acc[:]
                return AP(a.tensor, a.offset + q, [a.ap[0], [1, 1]])

            for t in range(T * reps):
                t = t % T
                vt = pv.tile([P, S * VSPAN], BF16)
                ut = pw.tile([P, S * M], BF16)
                mt = pw.tile([P, S * 3 * M], F32)
                for s in range(S):
                    i0 = t * S + s
                    nc.gpsimd.indirect_dma_start(
                        out=vt[:, s * VSPAN:(s + 1) * VSPAN],
                        out_offset=None, in_=v2[:, :],
                        in_offset=IndirectOffsetOnAxis(
                            ap=ov_all[:, i0:i0 + 1], axis=0))
                    nc.gpsimd.indirect_dma_start(
                        out=ut[:, s * M:(s + 1) * M],
                        out_offset=None, in_=u2[:, :],
                        in_offset=IndirectOffsetOnAxis(
                            ap=ou_all[:, i0:i0 + 1], axis=0))
                    nc.gpsimd.indirect_dma_start(
                        out=mt[:, s * 3 * M:(s + 1) * 3 * M],
                        out_offset=None, in_=m3[:, :],
                        in_offset=IndirectOffsetOnAxis(
                            ap=om_all[:, i0:i0 + 1], axis=0))

                vta, uta, mta = vt[:], ut[:], mt[:]

                def vap(off, *dims):
                    return AP(vta.tensor, vta.offset + off, [vta.ap[0], *dims])

                def uap(off, *dims):
                    return AP(uta.tensor, uta.offset + off, [uta.ap[0], *dims])

                def map_(off, *dims):
                    return AP(mta.tensor, mta.offset + off, [mta.ap[0], *dims])

                # fp32 copy of the U rows (feeds prefill, P1, P2 in fp32)
                uf = ps.tile([P, S * M], F32)
                ufa = uf[:]
                nc.vector.tensor_copy(ufa, uap(0, [M, S], [1, M]))

                def ufap(off, *dims):
                    return AP(ufa.tensor, ufa.offset + off, [ufa.ap[0], *dims])

                # dinv = 1/diag (diag at 64*r within each sample span)
                dinv = ps.tile([P, S * M], F32)
                dta = dinv[:]
                nc.vector.reciprocal(dta, vap(0, [VSPAN, S], [M, M]))
                ndinv = ps.tile([P, S * M], F32)
                nta = ndinv[:]
                nc.vector.tensor_scalar_mul(out=nta, in0=dta, scalar1=-1.0)

                def dap(off, *dims):
                    return AP(dta.tensor, dta.offset + off, [dta.ap[0], *dims])

                def nap(off, *dims):
                    return AP(nta.tensor, nta.offset + off, [nta.ap[0], *dims])

                # X: solution tile, lanes = [e: 0..S-1 | u: S..2S-1]
                # dualx: a second, one-element-left-shifted copy Xs lives in
                # the same tile at XSOFF (+2 guard) so odd-r mult streams are
                # 4B-aligned; every X write also writes Xs via a [DX,2] axis.
                XDT = BF16 if VARIANT['xdt'] == 'bf16' else F32
                dualx = VARIANT['dualx']
                XSOFF = 2 * S * M + 2
                DX = XSOFF - 1
                xsize = (XSOFF + 2 * S * M) if dualx else (2 * S * M)
                X = ps.tile([P, xsize], XDT)
                xta = X[:]

                def xap(off, *dims):
                    return AP(xta.tensor, xta.offset + off, [xta.ap[0], *dims])

                def dup(dims):
                    # prepend the dual-write axis when dualx is on
                    return ([DX, 2], *dims) if dualx else tuple(dims)

                def dup0(dims):
                    return ([0, 2], *dims) if dualx else tuple(dims)

                # prefill: e-half zeros except slot63 = ndinv[63];
                #          u-half = u * ndinv  (= -b*dinv)
                nc.scalar.memzero(xap(0, [1, xsize]))
                nc.vector.tensor_copy(xap(M - 1, *dup([[M, S], [1, 1]])),
                                      nap(M - 1, *dup0([[M, S], [1, 1]])))
                nc.vector.tensor_tensor(
                    out=xap(S * M, *dup([[M, S], [1, M]])),
                    in0=ufap(0, *dup0([[M, S], [1, M]])),
                    in1=nap(0, *dup0([[M, S], [1, M]])),
                    op=mult)
                # flip slot 63 of both halves: X[63] = +b[63]*dinv[63]
                nc.vector.tensor_scalar_mul(
                    out=xap(M - 1, *dup([[M, 2 * S], [1, 1]])),
                    in0=xap(M - 1, *dup([[M, 2 * S], [1, 1]])),
                    scalar1=-1.0)

                # products tile + brackets
                C = ps.tile([P, 2 * S * M], XDT)
                cta = C[:]

                def cap(off, *dims):
                    return AP(cta.tensor, cta.offset + off, [cta.ap[0], *dims])

                t2 = ps.tile([P, 2 * S], F32)
                t2a = t2[:]

                # backward substitution, r = 62..0
                for r in range(M - 2, -1, -1):
                    w = M - r   # stream length incl. diagonal seed
                    # in1: X[lane, r:64]; on odd r use the shifted copy so the
                    # stream start (bytes) is 4B-aligned for bf16 2x mode
                    if dualx and (r % 2 == 1):
                        x_in = XSOFF + r - 1
                    else:
                        x_in = r
                    # products: C[lane, 0:w] = A[r, r:64] * X[lane, r:64]
                    # (first product = diag * (-b*dinv) = -b[r])
                    if VARIANT['joint']:
                        nc.vector.tensor_tensor(
                            out=cap(0, [M, S], [S * M, 2], [1, w]),
                            in0=vap(M * r, [VSPAN, S], [0, 2], [1, w]),
                            in1=xap(x_in, [M, S], [S * M, 2], [1, w]),
                            op=mult)
                    else:
                        nc.vector.tensor_tensor(
                            out=cap(0, [M, S], [1, w]),
                            in0=vap(M * r, [VSPAN, S], [1, w]),
                            in1=xap(x_in, [M, S], [1, w]),
                            op=mult)
                        nc.vector.tensor_tensor(
                            out=cap(S * M, [M, S], [1, w]),
                            in0=vap(M * r, [VSPAN, S], [1, w]),
                            in1=xap(x_in + S * M, [M, S], [1, w]),
                            op=mult)
                    # bracket = -(sum stream) = b[r] - sum(A x)
                    nc.vector.tensor_reduce(
                        out=AP(t2a.tensor, t2a.offset, [t2a.ap[0], [1, 2 * S]]),
                        in_=cap(0, [M, 2 * S], [1, w]),
                        axis=mybir.AxisListType.X,
                        op=addop, negate=True)
                    # X[lane, r] = bracket * dinv[r]
                    nc.vector.tensor_tensor(
                        out=xap(r, *dup([[S * M, 2], [M, S]])),
                        in0=AP(t2a.tensor, t2a.offset,
                               [t2a.ap[0], *dup0([[S, 2], [1, S]])]),
                        in1=dap(r, *dup0([[0, 2], [M, S]])),
                        op=mult)

                # ---- epilogue on ACT + Pool (overlaps next tile's DVE) ----
                sc = ps.tile([P, S * M], F32)
                sca = sc[:]

                def scap(off, *dims):
                    return AP(sca.tensor, sca.offset + off, [sca.ap[0], *dims])

                sv = ps.tile([P, S], F32)
                sva = sv[:]
                sv_ap = AP(sva.tensor, sva.offset, [sva.ap[0], [1, S]])
                one = ps.tile([P, 4], F32)
                onea = one[:]

                def oneap(q):
                    return AP(onea.tensor, onea.offset + q, [onea.ap[0], [1, 1]])

                def acc_add(q, src):
                    nc.gpsimd.tensor_tensor(out=accslot(q), in0=accslot(q),
                                            in1=src, op=addop)

                # P1: sum ln(u_diag), sum ln(v_diag)  (separate slots 0/1)
                nc.scalar.activation(
                    out=sv_ap, in_=ufap(M - 1, [M, S], [1, 1]).squeeze(2),
                    func=mybir.ActivationFunctionType.Ln,
                    accum_out=oneap(0))
                acc_add(0, oneap(0))
                nc.scalar.activation(
                    out=sv_ap, in_=vap(M * (M - 1), [VSPAN, S], [1, 1]).squeeze(2),
                    func=mybir.ActivationFunctionType.Ln,
                    accum_out=oneap(1))
                acc_add(1, oneap(1))

                # P2: sum_s (sum_c u*(mean-mp))^2 ; m3 triples (m, mp, y)
                nc.gpsimd.tensor_tensor(
                    out=scap(0, [M, S], [1, M]),
                    in0=map_(0, [3 * M, S], [3, M]),
                    in1=map_(1, [3 * M, S], [3, M]),
                    op=subop)
                nc.gpsimd.tensor_tensor(
                    out=scap(0, [M, S], [1, M]),
                    in0=scap(0, [M, S], [1, M]),
                    in1=ufap(0, [M, S], [1, M]),
                    op=mult)
                nc.vector.tensor_reduce(
                    out=sv_ap, in_=scap(0, [M, S], [1, M]),
                    axis=mybir.AxisListType.X, op=addop)
                nc.scalar.activation(
                    out=sv_ap, in_=sv_ap,
                    func=mybir.ActivationFunctionType.Square,
                    accum_out=oneap(2))
                acc_add(2, oneap(2))

                # P3: sum ||x_u||^2
                nc.scalar.activation(
                    out=scap(0, [M, S], [1, M]),
                    in_=xap(S * M, [M, S], [1, M]),
                    func=mybir.ActivationFunctionType.Square,
                    accum_out=oneap(3))
                acc_add(3, oneap(3))

                # P5: sum ||x_e||^2
                nc.scalar.activation(
                    out=scap(0, [M, S], [1, M]),
                    in_=xap(0, [M, S], [1, M]),
                    func=mybir.ActivationFunctionType.Square,
                    accum_out=oneap(0))
                acc_add(5, oneap(0))

                # P4: sum (y[g]-mp[g])^2  (slots 3*63+2 and 3*63+1)
                nc.gpsimd.tensor_tensor(
                    out=sv_ap,
                    in0=map_(3 * (M - 1) + 2, [3 * M, S], [1, 1]).squeeze(2),
                    in1=map_(3 * (M - 1) + 1, [3 * M, S], [1, 1]).squeeze(2),
                    op=subop)
                nc.scalar.activation(
                    out=sv_ap, in_=sv_ap,
                    func=mybir.ActivationFunctionType.Square,
                    accum_out=oneap(1))
                acc_add(4, oneap(1))

            nc.sync.dma_start(out=out[:, :], in_=acc[:])
    return _split_multiwait(nc) if split else nc


def _build_padded_arrays(U_values, V_values, mean, mean_post, y, crow_u, crow_v):
    """Global (core-independent) host relayouts."""
    nnz = U_values.shape[0]
    # padded [N, 64] band of V in bf16
    rowlen = (crow_v[1:] - crow_v[:-1]).astype(np.int64)
    k = np.arange(M)
    idx = crow_v[:-1, None] + k[None, :]
    valid = k[None, :] < rowlen[:, None]
    v2 = np.zeros((N, M), dtype=np.float32)
    v2[valid] = V_values[np.clip(idx, 0, nnz - 1)[valid]]
    v2 = v2.reshape(-1)

    m3 = np.empty(3 * N, dtype=np.float32)
    m3[0::3] = mean
    m3[1::3] = mean_post
    m3[2::3] = y
    return v2, m3


def _prepare_core(v2_flat, U_values, m3_flat, mean, mean_post, y, g_core,
                  crow_u, crow_v, cap):
    """Per-core offsets + patch regions (numpy).  Returns None if the patch
    capacity is too small."""
    nnz = U_values.shape[0]
    g = g_core.astype(np.int64)
    irr = np.where(g < M - 1)[0]
    n_irr = len(irr)
    if n_irr > cap:
        return None

    base_v = np.where(g >= M - 1, M * (g - (M - 1)), 0)
    base_u = np.clip(crow_u[g + 1] - M, 0, None)
    base_m = np.where(g >= M - 1, 3 * (g - (M - 1)), 0)

    v_patch = np.zeros((cap, VSPAN), dtype=np.float32)
    u_patch = np.zeros((cap, M), dtype=np.float32)
    m_patch = np.zeros((cap, 3 * M), dtype=np.float32)
    for kk, b in enumerate(irr):
        gb = int(g[b])
        W = v_patch[kk].reshape(M, M)
        for r in range(M):
            J = gb - (M - 1) + r
            if J < 0:
                W[r, 0] = 1.0
            else:
                W[r, :] = v2_flat[M * J: M * (J + 1)]
        lu = int(crow_u[gb + 1] - crow_u[gb])
        u_patch[kk, M - lu:] = U_values[int(crow_u[gb + 1]) - lu:
                                        int(crow_u[gb + 1])]
        for r in range(M):
            J = gb - (M - 1) + r
            if J >= 0:
                m_patch[kk, 3 * r] = mean[J]
                m_patch[kk, 3 * r + 1] = mean_post[J]
                m_patch[kk, 3 * r + 2] = y[J]

    base_v[irr] = M * N + np.arange(n_irr, dtype=np.int64) * VSPAN
    base_u[irr] = nnz + np.arange(n_irr, dtype=np.int64) * M
    base_m[irr] = 3 * N + np.arange(n_irr, dtype=np.int64) * 3 * M

    Bc = len(g)
    T = Bc // (P * S)

    def pack(a):
        a = a.reshape(T, P, S).transpose(1, 0, 2).reshape(P, T * S)
        return np.ascontiguousarray(a).astype(np.int32)

    return dict(
        v_patch=v_patch.ravel(), u_patch=u_patch.ravel(), m_patch=m_patch.ravel(),
        offs_v=pack(base_v), offs_u=pack(base_u), offs_m=pack(base_m))


def _prep_all(inputs):
    U_values = np.asarray(inputs['U_values'], dtype=np.float32)
    V_values = np.asarray(inputs['V_values'], dtype=np.float32)
    mean = np.asarray(inputs['mean'], dtype=np.float32)
    mean_post = np.asarray(inputs['mean_post'], dtype=np.float32)
    y = np.asarray(inputs['y'], dtype=np.float32)
    mini_indices = np.asarray(inputs['mini_indices'], dtype=np.int32)
    crow_u = np.asarray(inputs['crow_u']).astype(np.int64)
    crow_v = np.asarray(inputs['crow_v']).astype(np.int64)

    B = mini_indices.shape[0]
    Bc = B // NCORES
    T = Bc // (P * S)
    v2_flat, m3_flat = _build_padded_arrays(
        U_values, V_values, mean, mean_post, y, crow_u, crow_v)

    cap = 16
    while True:
        preps = []
        ok = True
        for c in range(NCORES):
            g_c = np.sort(mini_indices[c * Bc:(c + 1) * Bc], kind='stable')
            pr = _prepare_core(v2_flat, U_values, m3_flat, mean, mean_post, y,
                               g_c, crow_u, crow_v, cap)
            if pr is None:
                ok = False
                break
            preps.append(pr)
        if ok:
            break
        cap *= 4

    # shared value arrays (+ per-core patches appended; patches are laid at
    # the same base offsets for every core, so arrays differ per core)
    in_maps = []
    for pr in preps:
        v2c = np.concatenate([v2_flat, pr['v_patch'],
                              np.zeros(VSPAN, np.float32)]).astype(BF_NP)
        u2c = np.concatenate([U_values, pr['u_patch'],
                              np.zeros(M, np.float32)]).astype(BF_NP)
        m3c = np.concatenate([m3_flat, pr['m_patch'],
                              np.zeros(3 * M, np.float32)]).astype(np.float32)
        in_maps.append({'v2': v2c[:, None], 'u2': u2c[:, None],
                        'm3': m3c[:, None],
                        'offs_v': pr['offs_v'], 'offs_u': pr['offs_u'],
                        'offs_m': pr['offs_m']})
    NV2 = in_maps[0]['v2'].shape[0]
    NU2 = in_maps[0]['u2'].shape[0]
    NM3 = in_maps[0]['m3'].shape[0]
    return in_maps, T, NV2, NU2, NM3


def kernel(U_values, V_values, mean, mean_post, y, noise, mini_indices,
           crow_u, crow_v):
    inputs = dict(U_values=U_values, V_values=V_values, mean=mean,
                  mean_post=mean_post, y=y, noise=noise,
                  mini_indices=mini_indices, crow_u=crow_u, crow_v=crow_v)
    noise = np.float32(np.asarray(noise))
    mini_indices = np.asarray(mini_indices, dtype=np.int32)
    B = mini_indices.shape[0]
    if B % (NCORES * P * S) != 0:
        return _host_fallback(
            np.asarray(U_values, np.float32), np.asarray(V_values, np.float32),
            np.asarray(mean, np.float32), np.asarray(mean_post, np.float32),
            np.asarray(y, np.float32), noise, mini_indices,
            np.asarray(crow_u).astype(np.int64),
            np.asarray(crow_v).astype(np.int64))

    in_maps, T, NV2, NU2, NM3 = _prep_all(inputs)
    key = (T, NV2, NU2, NM3)
    if key not in _cache:
        _cache[key] = _build_program(T, NV2, NU2, NM3)
    nc = _cache[key]

    res = run_bass_kernel_spmd(nc, in_maps, list(range(NCORES)))
    parts = np.zeros(8, dtype=np.float64)
    for c in range(NCORES):
        parts += res.results[c]['out'].astype(np.float64).sum(axis=0)
    lnU, lnV, P2, P3, P4, P5 = parts[:6]
    total = ((lnU - lnV) - 0.5 * P2 - 0.5 * P3
             - 0.5 * B * np.log(2.0 * np.pi * float(noise))
             - (P4 + P5) / (2.0 * float(noise)))
    return np.float32(total)


def build_for_bench(inputs, reps):
    """Build (nc, in_maps) with the device workload replicated `reps` times.
    Used only by bench.py, never by the grading path."""
    in_maps, T, NV2, NU2, NM3 = _prep_all(inputs)
    nc = _build_program(T, NV2, NU2, NM3, reps=reps)
    return nc, in_maps


def _host_fallback(U_values, V_values, mean, mean_post, y, noise,
                   mini_indices, crow_u, crow_v):
    """Numpy port of the reference; used only for off-spec batch sizes."""
    nnz = U_values.shape[0]
    g = mini_indices.astype(np.int64)
    L = np.minimum(g + 1, M)
    p = np.arange(M)
    valid = p[None, :] >= (M - L)[:, None]
    anc = g[:, None] - (M - 1 - p)[None, :]
    anc_c = np.clip(anc, 0, N - 1)
    u_idx = crow_u[g][:, None] + (p[None, :] - (M - L)[:, None])
    U_sub = np.where(valid, U_values[np.clip(u_idx, 0, nnz - 1)], 0.0)
    md = np.where(valid, (mean - mean_post)[anc_c], 0.0)
    jrow = anc_c[:, :, None]
    icol = anc_c[:, None, :]
    vidx = crow_v[jrow] + (icol - jrow)
    blk_mask = (valid[:, :, None] & valid[:, None, :]
                & (p[None, :, None] <= p[None, None, :]))
    eye = np.eye(M, dtype=np.float32)
    V_sub = np.where(blk_mask, V_values[np.clip(vidx, 0, nnz - 1)],
                     eye[None, :, :]).astype(np.float32)
    ej = np.zeros((len(g), M, 1), dtype=np.float32)
    ej[:, -1, 0] = 1.0
    sol_e = np.linalg.solve(V_sub, ej)
    marginalVarPost = np.sum(sol_e * sol_e, axis=(1, 2))
    sol_u = np.linalg.solve(V_sub, U_sub[:, :, None].astype(np.float32))
    innerCov = -0.5 * np.sum(sol_u * sol_u)
    innerMean = -0.5 * np.sum(np.sum(U_sub * md, axis=1) ** 2)
    logDet = (np.sum(np.log(U_values[crow_u[g + 1] - 1]))
              - np.sum(np.log(V_values[crow_v[g]])))
    Bn = len(g)
    resid = y[g] - mean_post[g]
    ell = (-0.5 * Bn * np.log(2.0 * np.pi * float(noise))
           - (np.sum(resid * resid) + np.sum(marginalVarPost))
           / (2.0 * float(noise)))
    return np.float32(logDet + innerMean + innerCov + ell)
